# revision 35
# baseline (speedup 1.0000x reference)
"""BiAttention similarity kernel for Trainium2, 8-core data-parallel over batch.

Computes, per batch b:
    s0 = c @ c_weight                  # [L, 1]
    s1 = (c @ q_weight)^T              # [1, L]
    s2 = (c * cq_weight) @ q^T         # [L, L]
    s  = s0 + s1 + s2 + bias           # [L, L]

Shapes (hardcoded): B=8, L=2048, D=256, fp32 in/out.

Distribution: data-parallel over batch, one batch per core. Host hands each
core its shard d-major (transposed) fp16 plus a packed per-partition weight
tile; the device returns s in fp16 (quantization ~5e-4 rel, well under
tolerance) and the host upcasts to fp32 — halving the dominant HBM write.

Device dataflow per core:
  - warmup: a few dummy matmuls while inputs stream in, so the PE HAM clock
    gate reaches 8/8 before real work starts.
  - inputs: weights + ct halves + qt halves ALL on the sync HWDGE ring in
    priority order — a single queue drains strictly FIFO at full rate, so
    each tensor completes as early as possible (two rings round-robin at
    packet granularity and finish everything late).
  - s1 broadcast computed directly: lhsT = qwb (q_weight replicated across
    all 128 output partitions) gives s1b[p, j] = sum_d qw[d] c^T[d, j] in
    one matmul pass; +bias folds into the fp16 PSUM->SBUF copies
    (per-partition AP bias, bias replicated down a wconsts column).
  - qmod = q^T * cq_weight on DVE, A columns first.
  - main loop over 16 row chunks: PE fills one [128,512] A tile + one
    [128,1536] B tile per chunk (2 K-chunks of 128), plus a tiny N=1 matmul
    per K-chunk computing s0 into a persistent PSUM column (~26 ns of issue
    each, rides the same LDWEIGHTS; bank pre-cleared once so these run
    start=False and don't fake-conflict with the s0 column reads).
  - drains: DVE scalar_tensor_tensor fuses (psumA + s0) + s1b -> fp16;
    ACT folds the s0 add into its fp32->fp16 copy of B (Identity + AP
    bias); DVE adds s1b over B in an all-fp16 2x-mode tensor_tensor.
  - one contiguous 512 KiB output DMA per chunk on the sync ring; the last
    chunk is drained B-first with split DMAs to shorten the tail.
"""

import numpy as np
from contextlib import ExitStack

import concourse.bass as bass
import concourse.tile as tile
from concourse import bacc, mybir
from concourse.bass_utils import run_bass_kernel_spmd

F32 = mybir.dt.float32
F16 = mybir.dt.float16
ADD = mybir.AluOpType.add

B = 8
L = 2048
D = 256
NK = D // 128          # 2 contraction chunks of 128
NI = L // 128          # 16 row chunks
ASPLIT = 512           # A = [0:512] (DVE fused drain), B = [512:2048] (ACT)
N_WARMUP = 9          # dummy matmuls to warm the PE clock gate

TRACE = False
LAST_RESULTS = None

_NC_CACHE = None


def build_body(ctx: ExitStack, tc: tile.TileContext, aps: dict):
    nc = tc.nc
    ct_d, qt_d, w_d, s_d = aps["ct"], aps["qt"], aps["wconsts"], aps["s"]
    Copy = mybir.ActivationFunctionType.Copy

    consts = ctx.enter_context(tc.tile_pool(name="consts", bufs=1))
    psA = ctx.enter_context(tc.tile_pool(name="psA", bufs=1, space="PSUM"))
    psB = ctx.enter_context(tc.tile_pool(name="psB", bufs=2, space="PSUM"))
    ps0 = ctx.enter_context(tc.tile_pool(name="ps0", bufs=1, space="PSUM"))
    outp = ctx.enter_context(tc.tile_pool(name="outp", bufs=6))

    # ---- constants -------------------------------------------------------
    # host-packed [128, 7] fp32: cols 0-1 cw(k0,k1), 2-3 qw, 4-5 cqw,
    # 6 bias replicated down all partitions
    # (memsets emitted first so the PE warmup isn't gated on the DMA)
    ones16 = consts.tile([1, 128], F16)
    nc.gpsimd.memset(ones16[0:1, :], 1.0)
    dummy16 = consts.tile([1, 512], F16)
    nc.gpsimd.memset(dummy16[0:1, :], 0.0)
    qwb = [consts.tile([128, 128], F16, tag=f"qwb{k}", name=f"qwb{k}")
           for k in range(NK)]
    for k in range(NK):
        nc.gpsimd.memset(qwb[k][:, :], 1.0)
    # wc loads FIRST on the sync HWDGE ring: per-ring FIFO drains its tiny
    # packets before the big ct/qt transfers, so the weights land in ~1 us
    # instead of losing the packet round-robin and finishing last
    wc = consts.tile([128, 7], F32)
    nc.sync.dma_start(wc[:], w_d)
    cw16 = consts.tile([128, NK], F16)
    nc.vector.tensor_copy(cw16[:], wc[:, 0:2])
    # qwb_k[d, m] = q_weight[d] for all m: broadcast via per-partition scale
    for k in range(NK):
        nc.vector.tensor_scalar_mul(qwb[k][:, :], qwb[k][:, :],
                                    wc[:, 2 + k:3 + k])

    # ---- PE warmup while inputs stream ----------------------------------
    warm = psA.tile([128, ASPLIT], F32, tag="A", name="warm")
    for w in range(N_WARMUP):
        nc.tensor.matmul(warm[:], ones16[0:1, :], dummy16[0:1, :],
                         start=True, stop=True)

    # ---- inputs: ALL on the sync ring, strict FIFO priority order --------
    # (one queue drains at full rate; splitting across two rings makes the
    # per-engine packet round-robin finish every tensor late)
    cT = [consts.tile([128, L], F16, tag=f"cT{k}", name=f"cT{k}")
          for k in range(NK)]
    qT = [consts.tile([128, L], F16, tag=f"qT{k}", name=f"qT{k}")
          for k in range(NK)]
    nc.sync.dma_start(cT[0][:, :], ct_d[0:128, :])
    nc.sync.dma_start(cT[1][:, :], ct_d[128:256, :])
    nc.sync.dma_start(qT[0][:, :], qt_d[0:128, :])
    nc.sync.dma_start(qT[1][:, :], qt_d[128:256, :])

    # ---- s1 broadcast, directly: s1b[p, j] = sum_d qw[d] * cT[d, j] ------
    # (lhsT = qwb so every output partition gets the same s1 row; no [1,L]
    # row stage, no separate broadcast matmul. The PSUM tiles are shaped
    # exactly like a chunk's A+B tiles so only one B slot is occupied and
    # the first chunk's B fill isn't blocked behind the s1b copies.)
    s1b16 = consts.tile([128, L], F16)
    s1psA = psA.tile([128, ASPLIT], F32, tag="A", name="s1psA")
    s1psB = psB.tile([128, L - ASPLIT], F32, tag="B", name="s1psB")
    for k in range(NK):
        nc.tensor.matmul(s1psA[:, :], qwb[k][:, :], cT[k][:, 0:ASPLIT],
                         start=(k == 0), stop=(k == NK - 1))
        for jj in range(3):
            nc.tensor.matmul(
                s1psB[:, jj * 512:(jj + 1) * 512], qwb[k][:, :],
                cT[k][:, ASPLIT + jj * 512:ASPLIT + (jj + 1) * 512],
                start=(k == 0), stop=(k == NK - 1))
    # ---- s0 columns, all up front ----------------------------------------
    # The 32 tiny N=1 matmuls (one per chunk per K-chunk) fill the PE's
    # input-wait gaps between ct1/qt arrivals: real work that keeps the HAM
    # activity window busy so the main loop starts at the warm 2.4 GHz
    # clock. One start=True matmul pre-clears the bank so they all run
    # start=False (per-element overwrite-then-accumulate, no whole-bank
    # has_written clears).
    s0c_ps = ps0.tile([128, NI], F32, tag="s0c", name="s0c_ps")
    s0_sb = consts.tile([128, NI], F32)
    nc.tensor.matmul(s0c_ps[:, :], ones16[0:1, :], dummy16[0:1, 0:NI],
                     start=True, stop=True)
    for k in range(NK):
        for i in range(NI):
            nc.tensor.matmul(s0c_ps[:, i:i + 1],
                             cT[k][:, i * 128:(i + 1) * 128],
                             cw16[:, k:k + 1], start=False, stop=(k == NK - 1),
                             skip_group_check=True)
    # qmod + s1b copies, DVE FIFO ordered by operand readiness: qA-k0
    # (qt0), the A-side s1b copy (ready early, frees the psA slot for the
    # first A fill), qA-k1 (qt1), the B scales, and the bulk s0 copy LAST —
    # it waits on all 32 tiny matmuls (~16.5us) and previously sat at the
    # head of the queue blocking the whole qmod chain behind it.
    # Both s1b copies on ACT, A first: psA frees at ~14.8 instead of ~16.5,
    # and DVE carries only the qmod scales, ordered so the k0 pair (gated
    # on the earlier qt0) runs before the k1 pair: the first chunk's B
    # fills unblock a full microsecond earlier.
    nc.scalar.add(s1b16[:, 0:ASPLIT], s1psA[:, :], wc[:, 6:7])
    nc.scalar.add(s1b16[:, ASPLIT:L], s1psB[:, :], wc[:, 6:7])
    nc.vector.tensor_scalar_mul(qT[0][:, 0:ASPLIT], qT[0][:, 0:ASPLIT],
                                wc[:, 4:5])
    nc.vector.tensor_scalar_mul(qT[0][:, ASPLIT:L], qT[0][:, ASPLIT:L],
                                wc[:, 4:5])
    nc.vector.tensor_scalar_mul(qT[1][:, 0:ASPLIT], qT[1][:, 0:ASPLIT],
                                wc[:, 5:6])
    nc.vector.tensor_scalar_mul(qT[1][:, ASPLIT:L], qT[1][:, ASPLIT:L],
                                wc[:, 5:6])
    # bulk s0 copy to SBUF once all columns are in
    nc.vector.tensor_copy(s0_sb[:, :], s0c_ps[:, :])

    # ---- main loop: 16 row chunks ----------------------------------------
    for i in range(NI):
        isl = slice(i * 128, (i + 1) * 128)
        last_chunk = (i == NI - 1)
        out_sb = outp.tile([128, L], F16, tag="out", name="out_sb")
        pa = psA.tile([128, ASPLIT], F32, tag="A", name="pa")
        pb = psB.tile([128, L - ASPLIT], F32, tag="B", name="pb")
        for k in range(NK):
            first, last = (k == 0), (k == NK - 1)
            # Chunks 0-1 fill A first: at startup qmod-A is ready before
            # qmod-B, so the A matmuls shouldn't queue behind B's gate.
            # All later chunks fill B first: pb completes one matmul-slot
            # earlier, the long ACT-B drain (which frees the psB slot two
            # chunks later) starts sooner, and the steady period stops
            # alternating on the slot wait.
            if i < 2:
                mms = [(pa, None)] + [(pb, jj) for jj in range(3)]
            else:
                mms = [(pb, jj) for jj in range(3)] + [(pa, None)]
            for ps, jj in mms:
                if jj is None:
                    nc.tensor.matmul(pa[:], cT[k][:, isl],
                                     qT[k][:, 0:ASPLIT],
                                     start=first, stop=last)
                else:
                    nc.tensor.matmul(pb[:, jj * 512:(jj + 1) * 512],
                                     cT[k][:, isl],
                                     qT[k][:, ASPLIT + jj * 512:
                                            ASPLIT + (jj + 1) * 512],
                                     start=first, stop=last)
        def drain_a():
            # A: one fused DVE op
            nc.vector.scalar_tensor_tensor(
                out_sb[:, 0:ASPLIT], pa[:], s0_sb[:, i:i + 1],
                s1b16[:, 0:ASPLIT], ADD, ADD)

        def drain_b():
            # B: ACT folds the s0 add into the fp32->fp16 copy, DVE adds
            # s1b in an all-fp16 2x-mode pass
            nc.scalar.add(out_sb[:, ASPLIT:L], pb[:], s0_sb[:, i:i + 1])
            nc.vector.tensor_add(out_sb[:, ASPLIT:L], out_sb[:, ASPLIT:L],
                                 s1b16[:, ASPLIT:L])

        if last_chunk:
            # tail-optimized: ship A as soon as its fused drain lands, and
            # drain/ship B in two region-aligned pieces so ACT, DVE and the
            # DMA pipeline instead of serializing on the full 1536 cols
            drain_a()
            nc.sync.dma_start(s_d[isl, 0:ASPLIT], out_sb[:, 0:ASPLIT])
            for lo, hi in ((ASPLIT, ASPLIT + 1024), (ASPLIT + 1024, L)):
                nc.scalar.add(out_sb[:, lo:hi], pb[:, lo - ASPLIT:hi - ASPLIT],
                              s0_sb[:, i:i + 1])
                nc.vector.tensor_add(out_sb[:, lo:hi], out_sb[:, lo:hi],
                                     s1b16[:, lo:hi])
                nc.sync.dma_start(s_d[isl, lo:hi], out_sb[:, lo:hi])
        else:
            # stt32 first on the DVE FIFO so the single-buffered A tile
            # frees before the next chunk's fill needs it
            drain_a()
            drain_b()
            nc.sync.dma_start(s_d[isl, :], out_sb[:, :])


def build_nc():
    nc = bacc.Bacc("TRN2", target_bir_lowering=False, debug=False)
    aps = {
        "ct": nc.dram_tensor("ct", [D, L], F16, kind="ExternalInput").ap(),
        "qt": nc.dram_tensor("qt", [D, L], F16, kind="ExternalInput").ap(),
        "wconsts": nc.dram_tensor("wconsts", [128, 7], F32,
                                  kind="ExternalInput").ap(),
        "s": nc.dram_tensor("s", [L, L], F16, kind="ExternalOutput").ap(),
    }
    with tile.TileContext(nc) as tc:
        with ExitStack() as ctx:
            build_body(ctx, tc, aps)
    nc.compile()
    return nc


def get_nc():
    global _NC_CACHE
    if _NC_CACHE is None:
        _NC_CACHE = build_nc()
    return _NC_CACHE


def kernel(c, q, c_weight, q_weight, cq_weight, bias):
    global LAST_RESULTS
    nc = get_nc()
    c = np.asarray(c, dtype=np.float32)
    q = np.asarray(q, dtype=np.float32)
    cw = np.asarray(c_weight, dtype=np.float32).reshape(2, 128).T  # [128, 2]
    qw = np.asarray(q_weight, dtype=np.float32).reshape(2, 128).T
    cqw = np.asarray(cq_weight, dtype=np.float32).reshape(2, 128).T
    bias = np.asarray(bias, dtype=np.float32)
    wconsts = np.zeros((128, 7), dtype=np.float32)
    wconsts[:, 0:2] = cw
    wconsts[:, 2:4] = qw
    wconsts[:, 4:6] = cqw
    wconsts[:, 6] = bias[0]
    in_maps = [
        {
            "ct": np.ascontiguousarray(c[b].T).astype(np.float16),
            "qt": np.ascontiguousarray(q[b].T).astype(np.float16),
            "wconsts": wconsts,
        }
        for b in range(B)
    ]
    res = run_bass_kernel_spmd(nc, in_maps, core_ids=list(range(B)), trace=TRACE)
    LAST_RESULTS = res
    return np.stack([res.results[b]["s"].astype(np.float32) for b in range(B)],
                    axis=0)


# revision 36
# speedup vs baseline: 1.0503x; 1.0503x over previous
"""BiAttention similarity kernel for Trainium2, 8-core data-parallel over batch.

Computes, per batch b:
    s0 = c @ c_weight                  # [L, 1]
    s1 = (c @ q_weight)^T              # [1, L]
    s2 = (c * cq_weight) @ q^T         # [L, L]
    s  = s0 + s1 + s2 + bias           # [L, L]

Shapes (hardcoded): B=8, L=2048, D=256, fp32 in/out.

Distribution: data-parallel over batch, one batch per core. Host hands each
core its shard d-major (transposed) fp16 plus a packed per-partition weight
tile; the device returns s in fp16 (quantization ~5e-4 rel, well under
tolerance) and the host upcasts to fp32 — halving the dominant HBM write.

Device dataflow per core:
  - warmup: a few dummy matmuls while inputs stream in, so the PE HAM clock
    gate reaches 8/8 before real work starts.
  - inputs: weights + ct halves + qt halves ALL on the sync HWDGE ring in
    priority order — a single queue drains strictly FIFO at full rate, so
    each tensor completes as early as possible (two rings round-robin at
    packet granularity and finish everything late).
  - s1 broadcast computed directly: lhsT = qwb (q_weight replicated across
    all 128 output partitions) gives s1b[p, j] = sum_d qw[d] c^T[d, j] in
    one matmul pass; +bias folds into the fp16 PSUM->SBUF copies
    (per-partition AP bias, bias replicated down a wconsts column).
  - qmod = q^T * cq_weight on DVE, A columns first.
  - main loop over 16 row chunks: PE fills one [128,512] A tile + one
    [128,1536] B tile per chunk (2 K-chunks of 128), plus a tiny N=1 matmul
    per K-chunk computing s0 into a persistent PSUM column (~26 ns of issue
    each, rides the same LDWEIGHTS; bank pre-cleared once so these run
    start=False and don't fake-conflict with the s0 column reads).
  - drains: DVE scalar_tensor_tensor fuses (psumA + s0) + s1b -> fp16;
    ACT folds the s0 add into its fp32->fp16 copy of B (Identity + AP
    bias); DVE adds s1b over B in an all-fp16 2x-mode tensor_tensor.
  - one contiguous 512 KiB output DMA per chunk on the sync ring; the last
    chunk is drained B-first with split DMAs to shorten the tail.
"""

import numpy as np
from contextlib import ExitStack

import concourse.bass as bass
import concourse.tile as tile
from concourse import bacc, mybir
from concourse.bass_utils import run_bass_kernel_spmd

F32 = mybir.dt.float32
F16 = mybir.dt.float16
ADD = mybir.AluOpType.add

B = 8
L = 2048
D = 256
NK = D // 128          # 2 contraction chunks of 128
NI = L // 128          # 16 row chunks
ASPLIT = 512           # A = [0:512] (DVE fused drain), B = [512:2048] (ACT)
N_WARMUP = 9          # dummy matmuls to warm the PE clock gate

TRACE = False
LAST_RESULTS = None

_NC_CACHE = None


def build_body(ctx: ExitStack, tc: tile.TileContext, aps: dict):
    nc = tc.nc
    ct_d, qt_d, w_d, s_d = aps["ct"], aps["qt"], aps["wconsts"], aps["s"]
    Copy = mybir.ActivationFunctionType.Copy

    consts = ctx.enter_context(tc.tile_pool(name="consts", bufs=1))
    psA = ctx.enter_context(tc.tile_pool(name="psA", bufs=1, space="PSUM"))
    psB = ctx.enter_context(tc.tile_pool(name="psB", bufs=2, space="PSUM"))
    ps0 = ctx.enter_context(tc.tile_pool(name="ps0", bufs=1, space="PSUM"))
    outp = ctx.enter_context(tc.tile_pool(name="outp", bufs=6))

    # ---- constants -------------------------------------------------------
    # host-packed [128, 7] fp32: cols 0-1 cw(k0,k1), 2-3 qw, 4-5 cqw,
    # 6 bias replicated down all partitions
    # (memsets emitted first so the PE warmup isn't gated on the DMA)
    ones16 = consts.tile([1, 128], F16)
    nc.gpsimd.memset(ones16[0:1, :], 1.0)
    dummy16 = consts.tile([1, 512], F16)
    nc.gpsimd.memset(dummy16[0:1, :], 0.0)
    qwb = [consts.tile([128, 128], F16, tag=f"qwb{k}", name=f"qwb{k}")
           for k in range(NK)]
    for k in range(NK):
        nc.gpsimd.memset(qwb[k][:, :], 1.0)
    # wc loads FIRST on the sync HWDGE ring: per-ring FIFO drains its tiny
    # packets before the big ct/qt transfers, so the weights land in ~1 us
    # instead of losing the packet round-robin and finishing last
    wc = consts.tile([128, 7], F32)
    nc.sync.dma_start(wc[:], w_d)
    cw16 = consts.tile([128, NK], F16)
    nc.vector.tensor_copy(cw16[:], wc[:, 0:2])
    # qwb_k[d, m] = q_weight[d] for all m: broadcast via per-partition scale
    for k in range(NK):
        nc.vector.tensor_scalar_mul(qwb[k][:, :], qwb[k][:, :],
                                    wc[:, 2 + k:3 + k])

    # ---- PE warmup while inputs stream ----------------------------------
    warm = psA.tile([128, ASPLIT], F32, tag="A", name="warm")
    for w in range(N_WARMUP):
        nc.tensor.matmul(warm[:], ones16[0:1, :], dummy16[0:1, :],
                         start=True, stop=True)

    # ---- inputs: ALL on the sync ring, strict FIFO priority order --------
    # (one queue drains at full rate; splitting across two rings makes the
    # per-engine packet round-robin finish every tensor late)
    cT = [consts.tile([128, L], F16, tag=f"cT{k}", name=f"cT{k}")
          for k in range(NK)]
    qT = [consts.tile([128, L], F16, tag=f"qT{k}", name=f"qT{k}")
          for k in range(NK)]
    nc.sync.dma_start(cT[0][:, :], ct_d[0:128, :])
    nc.sync.dma_start(cT[1][:, :], ct_d[128:256, :])
    nc.sync.dma_start(qT[0][:, :], qt_d[0:128, :])
    nc.sync.dma_start(qT[1][:, :], qt_d[128:256, :])

    # ---- s1 broadcast, directly: s1b[p, j] = sum_d qw[d] * cT[d, j] ------
    # (lhsT = qwb so every output partition gets the same s1 row; no [1,L]
    # row stage, no separate broadcast matmul. The PSUM tiles are shaped
    # exactly like a chunk's A+B tiles so only one B slot is occupied and
    # the first chunk's B fill isn't blocked behind the s1b copies.)
    s1b16 = consts.tile([128, L], F16)
    s1psA = psA.tile([128, ASPLIT], F32, tag="A", name="s1psA")
    s1psB = psB.tile([128, L - ASPLIT], F32, tag="B", name="s1psB")
    for k in range(NK):
        nc.tensor.matmul(s1psA[:, :], qwb[k][:, :], cT[k][:, 0:ASPLIT],
                         start=(k == 0), stop=(k == NK - 1))
        for jj in range(3):
            nc.tensor.matmul(
                s1psB[:, jj * 512:(jj + 1) * 512], qwb[k][:, :],
                cT[k][:, ASPLIT + jj * 512:ASPLIT + (jj + 1) * 512],
                start=(k == 0), stop=(k == NK - 1))
    # ---- s0 columns, all up front ----------------------------------------
    # The 32 tiny N=1 matmuls (one per chunk per K-chunk) fill the PE's
    # input-wait gaps between ct1/qt arrivals: real work that keeps the HAM
    # activity window busy so the main loop starts at the warm 2.4 GHz
    # clock. One start=True matmul pre-clears the bank so they all run
    # start=False (per-element overwrite-then-accumulate, no whole-bank
    # has_written clears).
    s0c_ps = ps0.tile([128, NI], F32, tag="s0c", name="s0c_ps")
    s0_sb = consts.tile([128, NI], F32)
    nc.tensor.matmul(s0c_ps[:, :], ones16[0:1, :], dummy16[0:1, 0:NI],
                     start=True, stop=True)
    for k in range(NK):
        for i in range(NI):
            nc.tensor.matmul(s0c_ps[:, i:i + 1],
                             cT[k][:, i * 128:(i + 1) * 128],
                             cw16[:, k:k + 1], start=False, stop=(k == NK - 1),
                             skip_group_check=True)
    # qmod + s1b copies, DVE FIFO ordered by operand readiness: qA-k0
    # (qt0), the A-side s1b copy (ready early, frees the psA slot for the
    # first A fill), qA-k1 (qt1), the B scales, and the bulk s0 copy LAST —
    # it waits on all 32 tiny matmuls (~16.5us) and previously sat at the
    # head of the queue blocking the whole qmod chain behind it.
    nc.vector.tensor_scalar_mul(qT[0][:, 0:ASPLIT], qT[0][:, 0:ASPLIT],
                                wc[:, 4:5])
    nc.vector.tensor_scalar_add(s1b16[:, 0:ASPLIT], s1psA[:, :], wc[:, 6:7])
    nc.scalar.add(s1b16[:, ASPLIT:L], s1psB[:, :], wc[:, 6:7])
    nc.vector.tensor_scalar_mul(qT[1][:, 0:ASPLIT], qT[1][:, 0:ASPLIT],
                                wc[:, 5:6])
    nc.vector.tensor_scalar_mul(qT[0][:, ASPLIT:L], qT[0][:, ASPLIT:L],
                                wc[:, 4:5])
    nc.vector.tensor_scalar_mul(qT[1][:, ASPLIT:L], qT[1][:, ASPLIT:L],
                                wc[:, 5:6])
    # bulk s0 copy to SBUF once all columns are in
    nc.vector.tensor_copy(s0_sb[:, :], s0c_ps[:, :])

    # ---- main loop: 16 row chunks ----------------------------------------
    for i in range(NI):
        isl = slice(i * 128, (i + 1) * 128)
        last_chunk = (i == NI - 1)
        out_sb = outp.tile([128, L], F16, tag="out", name="out_sb")
        pa = psA.tile([128, ASPLIT], F32, tag="A", name="pa")
        pb = psB.tile([128, L - ASPLIT], F32, tag="B", name="pb")
        for k in range(NK):
            first, last = (k == 0), (k == NK - 1)
            # Chunks 0-1 fill A first: at startup qmod-A is ready before
            # qmod-B, so the A matmuls shouldn't queue behind B's gate.
            # All later chunks fill B first: pb completes one matmul-slot
            # earlier, the long ACT-B drain (which frees the psB slot two
            # chunks later) starts sooner, and the steady period stops
            # alternating on the slot wait.
            if i < 2:
                mms = [(pa, None)] + [(pb, jj) for jj in range(3)]
            else:
                mms = [(pb, jj) for jj in range(3)] + [(pa, None)]
            for ps, jj in mms:
                if jj is None:
                    nc.tensor.matmul(pa[:], cT[k][:, isl],
                                     qT[k][:, 0:ASPLIT],
                                     start=first, stop=last)
                else:
                    nc.tensor.matmul(pb[:, jj * 512:(jj + 1) * 512],
                                     cT[k][:, isl],
                                     qT[k][:, ASPLIT + jj * 512:
                                            ASPLIT + (jj + 1) * 512],
                                     start=first, stop=last)
        def drain_a():
            # A: one fused DVE op
            nc.vector.scalar_tensor_tensor(
                out_sb[:, 0:ASPLIT], pa[:], s0_sb[:, i:i + 1],
                s1b16[:, 0:ASPLIT], ADD, ADD)

        def drain_b():
            # B: ACT folds the s0 add into the fp32->fp16 copy, DVE adds
            # s1b in an all-fp16 2x-mode pass
            nc.scalar.add(out_sb[:, ASPLIT:L], pb[:], s0_sb[:, i:i + 1])
            nc.vector.tensor_add(out_sb[:, ASPLIT:L], out_sb[:, ASPLIT:L],
                                 s1b16[:, ASPLIT:L])

        if last_chunk:
            # tail-optimized: ship A as soon as its fused drain lands, and
            # drain/ship B in two region-aligned pieces so ACT, DVE and the
            # DMA pipeline instead of serializing on the full 1536 cols
            drain_a()
            nc.sync.dma_start(s_d[isl, 0:ASPLIT], out_sb[:, 0:ASPLIT])
            for lo, hi in ((ASPLIT, ASPLIT + 1024), (ASPLIT + 1024, L)):
                nc.scalar.add(out_sb[:, lo:hi], pb[:, lo - ASPLIT:hi - ASPLIT],
                              s0_sb[:, i:i + 1])
                nc.vector.tensor_add(out_sb[:, lo:hi], out_sb[:, lo:hi],
                                     s1b16[:, lo:hi])
                nc.sync.dma_start(s_d[isl, lo:hi], out_sb[:, lo:hi])
        else:
            # stt32 first on the DVE FIFO so the single-buffered A tile
            # frees before the next chunk's fill needs it
            drain_a()
            drain_b()
            nc.sync.dma_start(s_d[isl, :], out_sb[:, :])


def build_nc():
    nc = bacc.Bacc("TRN2", target_bir_lowering=False, debug=False)
    aps = {
        "ct": nc.dram_tensor("ct", [D, L], F16, kind="ExternalInput").ap(),
        "qt": nc.dram_tensor("qt", [D, L], F16, kind="ExternalInput").ap(),
        "wconsts": nc.dram_tensor("wconsts", [128, 7], F32,
                                  kind="ExternalInput").ap(),
        "s": nc.dram_tensor("s", [L, L], F16, kind="ExternalOutput").ap(),
    }
    with tile.TileContext(nc) as tc:
        with ExitStack() as ctx:
            build_body(ctx, tc, aps)
    nc.compile()
    return nc


def get_nc():
    global _NC_CACHE
    if _NC_CACHE is None:
        _NC_CACHE = build_nc()
    return _NC_CACHE


def kernel(c, q, c_weight, q_weight, cq_weight, bias):
    global LAST_RESULTS
    nc = get_nc()
    c = np.asarray(c, dtype=np.float32)
    q = np.asarray(q, dtype=np.float32)
    cw = np.asarray(c_weight, dtype=np.float32).reshape(2, 128).T  # [128, 2]
    qw = np.asarray(q_weight, dtype=np.float32).reshape(2, 128).T
    cqw = np.asarray(cq_weight, dtype=np.float32).reshape(2, 128).T
    bias = np.asarray(bias, dtype=np.float32)
    wconsts = np.zeros((128, 7), dtype=np.float32)
    wconsts[:, 0:2] = cw
    wconsts[:, 2:4] = qw
    wconsts[:, 4:6] = cqw
    wconsts[:, 6] = bias[0]
    in_maps = [
        {
            "ct": np.ascontiguousarray(c[b].T).astype(np.float16),
            "qt": np.ascontiguousarray(q[b].T).astype(np.float16),
            "wconsts": wconsts,
        }
        for b in range(B)
    ]
    res = run_bass_kernel_spmd(nc, in_maps, core_ids=list(range(B)), trace=TRACE)
    LAST_RESULTS = res
    return np.stack([res.results[b]["s"].astype(np.float32) for b in range(B)],
                    axis=0)
